# revision 1
# baseline (speedup 1.0000x reference)
"""Causal multi-head attention on 8 Trainium2 NeuronCores.

Strategy: tensor-parallel over heads (16 heads / 8 cores = 2 heads per core).
Each core receives the full activations x^T (bf16, [d_model, B*S]) plus its
column-shard of Wq/Wk/Wv ([1024, 128]) and row-shard of Wo ([128, 1024]).
It computes Q^T/K^T in [feat, token] layout, V in [token, feat] layout
(with an appended ones-column so the attention-weight row-sums fall out of
the same PSUM accumulation as A@V), runs causal attention in "transposed"
layout (scores^T = K^T.T @ Q^T -> [k, q]) with exp on the scalar engine and
no max-subtraction (scores are O(6) here so exp never overflows), then
multiplies by its Wo shard to produce a partial [B*S, 1024] output.
The host sums the 8 partials and adds bo.  No collectives are needed.
"""

import os
import sys

for p in ("/opt/trn_rl_repo", "/root/.axon_site/_ro/trn_rl_repo"):
    if os.path.isdir(p) and p not in sys.path:
        sys.path.append(p)

import numpy as np
import ml_dtypes

import concourse.bass as bass
import concourse.bacc as bacc
import concourse.mybir as mybir
import concourse.tile as tile
from concourse.bass_utils import run_bass_kernel_spmd

BF16 = mybir.dt.bfloat16
F32 = mybir.dt.float32
NP_BF16 = ml_dtypes.bfloat16

D_MODEL = 1024
NUM_HEADS = 16
D_K = 64
B = 2
S = 2048
T = B * S            # 4096 tokens
N_CORES = 8
FPC = 128            # features per core (2 heads x 64)
N_QT = S // 512      # 4 q-tiles of 512 per batch
N_KT = S // 128      # 16 k-tiles of 128 per batch
KC = D_MODEL // 128  # 8 contraction chunks for the projections

_AluOp = mybir.AluOpType
_Act = mybir.ActivationFunctionType


def build_nc():
    nc = bacc.Bacc()

    xT = nc.declare_dram_parameter("xT", [D_MODEL, T], BF16, isOutput=False)
    wq = nc.declare_dram_parameter("wq", [D_MODEL, FPC], BF16, isOutput=False)
    wk = nc.declare_dram_parameter("wk", [D_MODEL, FPC], BF16, isOutput=False)
    wv = nc.declare_dram_parameter("wv", [D_MODEL, FPC], BF16, isOutput=False)
    wo = nc.declare_dram_parameter("wo", [FPC, D_MODEL], BF16, isOutput=False)
    bq = nc.declare_dram_parameter("bq", [FPC, 512], F32, isOutput=False)
    bk = nc.declare_dram_parameter("bk", [FPC, 512], F32, isOutput=False)
    bv = nc.declare_dram_parameter("bv", [1, FPC], F32, isOutput=False)
    masks = nc.declare_dram_parameter("masks", [4, 128, 512], BF16, isOutput=False)
    out = nc.declare_dram_parameter("out", [T, D_MODEL], F32, isOutput=True)

    with tile.TileContext(nc) as tc:
        with (
            tc.tile_pool(name="persist", bufs=1) as persist,
            tc.tile_pool(name="cc_pool", bufs=3) as cc_pool,
            tc.tile_pool(name="at_pool", bufs=6) as at_pool,
            tc.tile_pool(name="tmp_pool", bufs=2) as tmp_pool,
            tc.tile_pool(name="ob_pool", bufs=4) as ob_pool,
        ):
            # ---------- load everything ----------

            def load_w(dram, tag):
                t_ = persist.tile([128, KC, FPC], BF16, tag=tag, name=tag)
                nc.gpsimd.dma_start(
                    out=t_, in_=dram.rearrange("(c p) f -> p c f", p=128)
                )
                return t_

            wq_sb = load_w(wq, "wq")
            wk_sb = load_w(wk, "wk")
            wv_sb = load_w(wv, "wv")

            wo_sb = persist.tile([128, D_MODEL], BF16, tag="wo")
            nc.sync.dma_start(out=wo_sb, in_=wo[:, :])

            bq_sb = persist.tile([128, 512], F32, tag="bq")
            nc.sync.dma_start(out=bq_sb, in_=bq[:, :])
            bk_sb = persist.tile([128, 512], F32, tag="bk")
            nc.sync.dma_start(out=bk_sb, in_=bk[:, :])
            bv_sb = persist.tile([128, FPC], F32, tag="bv")
            nc.gpsimd.dma_start(out=bv_sb, in_=bv.ap().to_broadcast([128, FPC]))

            mask_sb = []
            for r in range(4):
                t_ = persist.tile([128, 512], BF16, tag=f"mask{r}", name=f"mask{r}")
                nc.gpsimd.dma_start(out=t_, in_=masks[r, :, :])
                mask_sb.append(t_)

            xt = [[persist.tile([128, S], BF16, tag=f"xt{c}_{bb}", name=f"xt{c}_{bb}")
                   for bb in range(B)] for c in range(KC)]
            for bb in range(B):
                for c in range(KC):
                    nc.sync.dma_start(
                        out=xt[c][bb],
                        in_=xT[c * 128:(c + 1) * 128, bb * S:(bb + 1) * S],
                    )

            ones_sb = persist.tile([128, 64], F32, tag="ones")
            nc.vector.memset(ones_sb, 1.0)
            rtile = persist.tile([128, 512], F32, tag="rtile")
            nc.vector.memset(rtile, 0.0)
            qt_tiles = [persist.tile([128, 512], BF16, tag=f"qt{i}", name=f"qt{i}")
                        for i in range(T // 512)]
            kt_tiles = [persist.tile([128, 512], BF16, tag=f"kt{i}", name=f"kt{i}")
                        for i in range(T // 512)]
            v_sb = [persist.tile([128, 130], BF16, tag=f"v{g}", name=f"v{g}")
                    for g in range(T // 128)]
            for g in range(T // 128):
                nc.vector.memset(v_sb[g][:, 64:65], 1.0)
                nc.vector.memset(v_sb[g][:, 129:130], 1.0)

            with (
                tc.tile_pool(name="ps", bufs=2, space="PSUM") as ps,
                tc.tile_pool(name="rpool", bufs=2) as rpool,
            ):
                def proj_q(ti):
                    bb, loc = ti // 4, (ti % 4) * 512
                    sl = slice(loc, loc + 512)
                    pq = ps.tile([128, 512], F32, tag="po", name="pq")
                    for c in range(KC):
                        nc.tensor.matmul(
                            pq, lhsT=wq_sb[:, c, :], rhs=xt[c][bb][:, sl],
                            start=(c == 0), stop=(c == KC - 1),
                        )
                    nc.vector.tensor_tensor(
                        out=qt_tiles[ti], in0=pq, in1=bq_sb, op=_AluOp.add,
                    )

                def proj_k(ti):
                    bb, loc = ti // 4, (ti % 4) * 512
                    sl = slice(loc, loc + 512)
                    pk = ps.tile([128, 512], F32, tag="po", name="pk")
                    for c in range(KC):
                        nc.tensor.matmul(
                            pk, lhsT=wk_sb[:, c, :], rhs=xt[c][bb][:, sl],
                            start=(c == 0), stop=(c == KC - 1),
                        )
                    nc.vector.tensor_tensor(
                        out=kt_tiles[ti], in0=pk, in1=bk_sb, op=_AluOp.add,
                    )

                def proj_v(g):
                    bb, loc = g // N_KT, (g % N_KT) * 128
                    gsl = slice(loc, loc + 128)
                    pv = ps.tile([128, 512], F32, tag="po", name="pv")
                    for c in range(KC):
                        nc.tensor.matmul(
                            pv[:, 0:FPC], lhsT=xt[c][bb][:, gsl], rhs=wv_sb[:, c, :],
                            start=(c == 0), stop=(c == KC - 1),
                        )
                    for h in range(2):
                        nc.vector.tensor_tensor(
                            out=v_sb[g][:, h * 65:h * 65 + 64],
                            in0=pv[:, h * 64:(h + 1) * 64],
                            in1=bv_sb[:, h * 64:(h + 1) * 64],
                            op=_AluOp.add,
                        )

                def attention(b, qt, fillers=()):
                    fillers = list(fillers)
                    tok0 = b * S + qt * 512
                    cc = cc_pool.tile([128, 512], BF16, tag="cc", name="cc")
                    for h in range(2):
                        hsl = slice(h * 64, (h + 1) * 64)
                        av = ps.tile([65, 512], F32, tag="av", name="av")
                        nk = (qt + 1) * 4     # visible 128-k-tiles
                        ng = nk // 2          # exp groups of 2 k-tiles
                        for grp in range(ng):
                            sc = ps.tile([128, 1024], F32, tag="sc", name="sc")
                            at = at_pool.tile([128, 1024], BF16, tag="at", name="at")
                            for j in range(2):
                                ki = grp * 2 + j
                                kt_i = b * 4 + ki // 4
                                ko = (ki % 4) * 128
                                nc.tensor.matmul(
                                    sc[:, j * 512:(j + 1) * 512],
                                    lhsT=kt_tiles[kt_i][hsl, ko:ko + 128],
                                    rhs=qt_tiles[b * 4 + qt][hsl, :],
                                    start=True, stop=True,
                                )
                            nc.scalar.activation(out=at, in_=sc, func=_Act.Exp)
                            for j in range(2):
                                ki = grp * 2 + j
                                rel = ki * 128 - qt * 512
                                if rel >= 0:
                                    nc.vector.tensor_mul(
                                        at[:, j * 512:(j + 1) * 512],
                                        at[:, j * 512:(j + 1) * 512],
                                        mask_sb[rel // 128],
                                    )
                            for j in range(2):
                                ki = grp * 2 + j
                                g = b * N_KT + ki
                                nc.tensor.matmul(
                                    av,
                                    lhsT=v_sb[g][:, h * 65:h * 65 + 65],
                                    rhs=at[:, j * 512:(j + 1) * 512],
                                    start=(ki == 0), stop=(ki == nk - 1),
                                )
                            if fillers:
                                fillers.pop(0)()
                        nc.vector.reciprocal(rtile[64:65, :], av[64:65, :])
                        bc = ps.tile([64, 512], F32, tag="av", name="bc")
                        nc.tensor.matmul(bc, lhsT=ones_sb, rhs=rtile,
                                         start=True, stop=True)
                        bc_sb = rpool.tile([64, 512], F32, tag="bcs", name="bcs")
                        nc.vector.tensor_copy(bc_sb, bc)
                        if h == 0:
                            nc.vector.tensor_tensor(
                                out=cc[0:64, :], in0=av[0:64, :], in1=bc_sb,
                                op=_AluOp.mult,
                            )
                        else:
                            h1t = tmp_pool.tile([64, 512], BF16, tag="h1t", name="h1t")
                            nc.vector.tensor_tensor(
                                out=h1t, in0=av[0:64, :], in1=bc_sb,
                                op=_AluOp.mult,
                            )
                            nc.gpsimd.dma_start(out=cc[64:128, :], in_=h1t)
                    for ot in range(4):
                        for n2 in range(2):
                            po = ps.tile([128, 512], F32, tag="po", name="po")
                            nc.tensor.matmul(
                                po,
                                lhsT=cc[:, ot * 128:(ot + 1) * 128],
                                rhs=wo_sb[:, n2 * 512:(n2 + 1) * 512],
                                start=True, stop=True,
                            )
                            ob = ob_pool.tile([128, 512], F32, tag="ob", name="ob")
                            if b == 1 and n2 == 0:
                                nc.scalar.copy(ob, po)
                            else:
                                nc.vector.tensor_copy(ob, po)
                            nc.sync.dma_start(
                                out=out[tok0 + ot * 128: tok0 + (ot + 1) * 128,
                                        n2 * 512:(n2 + 1) * 512],
                                in_=ob,
                            )

                # batch-0 projections
                for ti in range(4):
                    proj_q(ti)
                    proj_k(ti)
                for g in range(N_KT):
                    proj_v(g)
                # batch-1 projection work, dripped into batch-0 attention one
                # unit per exp-group so PE fills ACT-bound gaps
                fillers = []
                for ti in range(4, 8):
                    fillers.append(lambda ti=ti: proj_q(ti))
                    fillers.append(lambda ti=ti: proj_k(ti))
                for g in range(N_KT, 2 * N_KT):
                    fillers.append(lambda g=g: proj_v(g))
                n_groups = [20, 14, 8, 4]  # remaining groups at qt=3,2,1,0 (x2 heads)
                it = iter(fillers)
                rem = fillers[:]
                for qt in [3, 2, 1, 0]:
                    take = min(len(rem), {3: 10, 2: 8, 1: 4, 0: 2}[qt])
                    attention(0, qt, rem[:take])
                    rem = rem[take:]
                for f in rem:
                    f()
                for qt in [3, 2, 1, 0]:
                    attention(1, qt)
    return nc


_NC_CACHE = None


def _get_nc():
    global _NC_CACHE
    if _NC_CACHE is None:
        _NC_CACHE = build_nc()
        if not _NC_CACHE.is_finalized():
            _NC_CACHE.finalize()
    return _NC_CACHE


def _make_masks():
    p = np.arange(128)[:, None]
    f = np.arange(512)[None, :]
    m = np.zeros((4, 128, 512), NP_BF16)
    for rel in range(4):
        m[rel] = (p + 128 * rel <= f).astype(NP_BF16)
    return m


def _shard_inputs(x, Wq, bq, Wk, bk, Wv, bv, Wo, bo):
    x = np.asarray(x, np.float32)
    Wq, Wk, Wv, Wo = (np.asarray(a, np.float32) for a in (Wq, Wk, Wv, Wo))
    bq, bk, bv = (np.asarray(a, np.float32) for a in (bq, bk, bv))

    xT = np.ascontiguousarray(x.reshape(T, D_MODEL).T).astype(NP_BF16)
    masks = _make_masks()

    in_maps = []
    for c in range(N_CORES):
        fs = slice(c * FPC, (c + 1) * FPC)
        in_maps.append({
            "xT": xT,
            "wq": np.ascontiguousarray(Wq[:, fs] / 8.0).astype(NP_BF16),
            "wk": np.ascontiguousarray(Wk[:, fs]).astype(NP_BF16),
            "wv": np.ascontiguousarray(Wv[:, fs]).astype(NP_BF16),
            "wo": np.ascontiguousarray(Wo[fs, :]).astype(NP_BF16),
            "bq": np.ascontiguousarray(
                np.broadcast_to((bq[fs] / 8.0)[:, None], (FPC, 512))),
            "bk": np.ascontiguousarray(
                np.broadcast_to(bk[fs][:, None], (FPC, 512))),
            "bv": np.ascontiguousarray(bv[fs]).reshape(1, FPC),
            "masks": masks,
        })
    return in_maps


def _gather(results, bo):
    total = np.zeros((T, D_MODEL), np.float32)
    for c in range(N_CORES):
        total += np.asarray(results[c]["out"], np.float32)
    total += np.asarray(bo, np.float32)[None, :]
    return total.reshape(B, S, D_MODEL)


def kernel(x, Wq, bq, Wk, bk, Wv, bv, Wo, bo):
    in_maps = _shard_inputs(x, Wq, bq, Wk, bk, Wv, bv, Wo, bo)
    nc = _get_nc()
    res = run_bass_kernel_spmd(nc, in_maps, list(range(N_CORES)))
    return _gather(res.results, bo)


if __name__ == "__main__":
    rng = np.random.default_rng(0)
    x = rng.standard_normal((B, S, D_MODEL)).astype(np.float32)
    sc = 1 / np.sqrt(D_MODEL)
    args = dict(
        x=x,
        Wq=rng.standard_normal((D_MODEL, D_MODEL)).astype(np.float32) * sc,
        bq=np.zeros(D_MODEL, np.float32),
        Wk=rng.standard_normal((D_MODEL, D_MODEL)).astype(np.float32) * sc,
        bk=np.zeros(D_MODEL, np.float32),
        Wv=rng.standard_normal((D_MODEL, D_MODEL)).astype(np.float32) * sc,
        bv=np.zeros(D_MODEL, np.float32),
        Wo=rng.standard_normal((D_MODEL, D_MODEL)).astype(np.float32) * sc,
        bo=np.zeros(D_MODEL, np.float32),
    )
    out = kernel(**args)
    print("kernel output", out.shape, out.dtype, np.abs(out).max())

